# revision 1
# baseline (speedup 1.0000x reference)
"""DeepseekV4-style sparse attention on 8 Trainium2 cores (Bass/Tile).

Sharding: data-parallel over batch (2) x tensor-parallel over heads (16 -> 4
groups of 4).  Core c handles batch c//4 and heads [4*(c%4), 4*(c%4)+4).
wkv/wgate/ape (single shared KV head) are replicated; each core computes the
pooled KV itself.  Per-core partial outputs (attn_heads @ wo_rows) are summed
on the host.

Device layout notes:
  - host passes hidden TRANSPOSED ([HID, S]) and cast to bf16 so every matmul
    has its contraction dim on partitions with no on-device transposes
  - q is produced directly in qT layout [head_dim, t] (matmul lhsT = wq)
  - scores are computed transposed (S^T[w, q]); softmax sums over w via a
    ones-vector matmul, PV consumes exp(S^T) directly, and the resulting
    attnT [hd, t] is exactly the lhsT the output projection wants
  - causal structure: query chunk j (512 queries) sees w-chunks 0..j; only
    the diagonal chunk needs a mask, identical for every j (precomputed 0/1)
  - all RoPE math runs on partitions 64..127 so no op shifts partition bases
"""

import numpy as np
import ml_dtypes

import concourse.bass as bass
import concourse.mybir as mybir
import concourse.tile as tile
from concourse.bass import ts
from concourse.masks import make_identity

F32 = mybir.dt.float32
BF16 = mybir.dt.bfloat16
AF = mybir.ActivationFunctionType

# Problem constants (hardcoded per the harness contract).
B, S, HID, NH, HD, RD, RATIO = 2, 4096, 2048, 16, 128, 64, 4
THETA = 10000.0
NW = S // RATIO              # 1024 pooled windows
N_CORES = 8
HPC = 4                      # heads per core
CW = HPC * HD                # per-core q/wo width (512)
TCH = 512                    # t-chunk size
NCH = S // TCH               # 8 t-chunks
WCH = 128                    # w-chunk size
KCH = HID // 128             # 16 contraction chunks
SCALE = HD ** -0.5

_PAIR_SWAP = [i ^ 1 for i in range(32)]


def _build_nc(n_reps: int = 1, split_waits: bool = True):
    nc = bass.Bass()
    dp = nc.declare_dram_parameter
    ht = dp("ht", [HID, S], BF16, isOutput=False)
    wq = dp("wq", [HID, CW], BF16, isOutput=False)
    wkv = dp("wkv", [HID, 2 * HD], BF16, isOutput=False)
    wg = dp("wg", [HID, 2 * HD], BF16, isOutput=False)
    wo = dp("wo", [CW, HID], BF16, isOutput=False)
    eape = dp("eape", [HD, 2 * RATIO], F32, isOutput=False)
    esink = dp("esink", [1, HPC], F32, isOutput=False)
    cosq = dp("cosq", [RD, S], BF16, isOutput=False)
    sinq = dp("sinq", [RD, S], BF16, isOutput=False)
    cosk = dp("cosk", [RD, NW], BF16, isOutput=False)
    sink = dp("sink", [RD, NW], BF16, isOutput=False)
    bandm = dp("bandm", [WCH, TCH], BF16, isOutput=False)
    out = dp("out", [S, HID], F32, isOutput=True)
    args = (ht, wq, wkv, wg, wo, eape, esink, cosq, sinq, cosk, sink, bandm, out)

    with tile.TileContext(nc) as tc:
        if n_reps > 1:
            with tc.For_i(0, n_reps, 1):
                _body(nc, tc, *args)
        else:
            _body(nc, tc, *args)
    if split_waits:
        _split_multi_waits(nc)
    return nc


def _body(nc, tc, ht, wq, wkv, wg, wo, eape, esink,
          cosq, sinq, cosk, sink, bandm, out):
    with tc.tile_pool(name="persist", bufs=1) as pp:
        # ---- persistent SBUF state ----
        qT = [pp.tile([128, S], BF16, tag=f"qT{m}", name=f"qT{m}") for m in range(HPC)]
        kvlo = pp.tile([HD, RATIO + S], BF16, tag="kvlo", name="kvlo")
        kvhi = pp.tile([HD, S], BF16, tag="kvhi", name="kvhi")
        glo = pp.tile([HD, RATIO + S], BF16, tag="glo", name="glo")
        ghi = pp.tile([HD, S], BF16, tag="ghi", name="ghi")
        # rope tables live on partitions 64..127 (matching the rope rows)
        cosq_s = pp.tile([128, S], BF16, tag="cosq", name="cosq")
        sinq_s = pp.tile([128, S], BF16, tag="sinq", name="sinq")
        cosk_s = pp.tile([128, NW], BF16, tag="cosk", name="cosk")
        sink_s = pp.tile([128, NW], BF16, tag="sink", name="sink")
        eape_s = pp.tile([HD, 2 * RATIO], F32, tag="eape", name="eape")
        esink_s = pp.tile([1, HPC], F32, tag="esink", name="esink")
        bandm_s = pp.tile([WCH, TCH], BF16, tag="bandm", name="bandm")
        wo_s = pp.tile([HD, HPC, HID], BF16, tag="wo", name="wo")
        ones_w = pp.tile([WCH, 1], BF16, tag="ones_w", name="ones_w")
        ones_p = pp.tile([1, HD], F32, tag="ones_p", name="ones_p")
        kT = pp.tile([HD, NW], BF16, tag="kT", name="kT")
        v_s = pp.tile([WCH, NW // WCH, HD], BF16, tag="v", name="v")
        pooledT = pp.tile([HD, NW], F32, tag="pooledT", name="pooledT")
        ident = pp.tile([128, 128], F32, tag="ident", name="ident")

        nc.sync.dma_start(cosq_s[RD:128, :], cosq[:])
        nc.sync.dma_start(sinq_s[RD:128, :], sinq[:])
        nc.sync.dma_start(cosk_s[RD:128, :], cosk[:])
        nc.sync.dma_start(sink_s[RD:128, :], sink[:])
        nc.sync.dma_start(eape_s[:], eape[:])
        nc.sync.dma_start(esink_s[:], esink[:])
        nc.sync.dma_start(bandm_s[:], bandm[:])
        nc.sync.dma_start(wo_s[:], wo.rearrange("(h p) e -> p h e", p=HD))
        nc.vector.memset(ones_w[:], 1.0)
        nc.vector.memset(ones_p[:], 1.0)
        nc.gpsimd.memset(kvlo[:, 0:RATIO], 0.0)
        nc.gpsimd.memset(glo[:, 0:RATIO], -30000.0)
        make_identity(nc, ident[:])

        # ---- stage A: projections (q with fused RoPE, kv, gate) ----
        with (
            tc.tile_pool(name="wts", bufs=1) as wts,
            tc.tile_pool(name="hslab", bufs=2) as hs,
            tc.tile_pool(name="evict", bufs=3) as ev,
            tc.tile_pool(name="psA", bufs=6, space="PSUM") as psA,
        ):
            wq_s = wts.tile([128, KCH, CW], BF16, tag="wq", name="wq")
            wkv_s = wts.tile([128, KCH, 2 * HD], BF16, tag="wkv", name="wkv")
            wg_s = wts.tile([128, KCH, 2 * HD], BF16, tag="wg", name="wg")
            nc.sync.dma_start(wq_s[:], wq.rearrange("(k p) c -> p k c", p=128))
            nc.sync.dma_start(wkv_s[:], wkv.rearrange("(k p) c -> p k c", p=128))
            nc.sync.dma_start(wg_s[:], wg.rearrange("(k p) c -> p k c", p=128))

            for j in range(NCH):
                tsl = ts(j, TCH)
                hsl = hs.tile([128, KCH, TCH], BF16, tag="hslab", name="hslab")
                nc.sync.dma_start(
                    hsl[:], ht[:, tsl].rearrange("(k p) t -> p k t", p=128))

                # q projection, one head (=one 128-col chunk of wq) at a time
                for m in range(HPC):
                    ps = psA.tile([128, TCH], F32, tag="psA", name="psA")
                    for k in range(KCH):
                        nc.tensor.matmul(ps[:], wq_s[:, k, ts(m, 128)],
                                         hsl[:, k, :], start=(k == 0),
                                         stop=(k == KCH - 1))
                    # rows 0:64 pass through; rows 64:128 interleaved RoPE
                    nc.scalar.copy(qT[m][0:RD, tsl], ps[0:RD, :])
                    rb = ev.tile([128, TCH], BF16, tag="ropebuf", name="ropebuf")
                    nc.scalar.copy(rb[RD:128, :], ps[RD:128, :])
                    sw = ev.tile([128, TCH], BF16, tag="ropeswap", name="ropeswap")
                    nc.vector.stream_shuffle(sw[RD:128, :], rb[RD:128, :],
                                             _PAIR_SWAP)
                    t1 = ev.tile([128, TCH], BF16, tag="ropet1", name="ropet1")
                    nc.vector.tensor_mul(t1[RD:128, :], rb[RD:128, :],
                                         cosq_s[RD:128, tsl])
                    t2 = ev.tile([128, TCH], BF16, tag="ropet2", name="ropet2")
                    nc.vector.tensor_mul(t2[RD:128, :], sw[RD:128, :],
                                         sinq_s[RD:128, tsl])
                    nc.vector.tensor_add(qT[m][RD:128, tsl], t1[RD:128, :],
                                         t2[RD:128, :])

                # kv / gate projections (lo = features 0:128, hi = 128:256)
                for dst, lo, wsrc in (
                    (kvlo, True, wkv_s), (kvhi, False, wkv_s),
                    (glo, True, wg_s), (ghi, False, wg_s),
                ):
                    ps = psA.tile([128, TCH], F32, tag="psA", name="psA")
                    col = slice(0, 128) if lo else slice(128, 256)
                    for k in range(KCH):
                        nc.tensor.matmul(ps[:], wsrc[:, k, col], hsl[:, k, :],
                                         start=(k == 0), stop=(k == KCH - 1))
                    if lo:
                        nc.scalar.copy(
                            dst[:, RATIO + j * TCH:RATIO + (j + 1) * TCH], ps[:])
                    else:
                        nc.scalar.copy(dst[:, tsl], ps[:])

        # ---- stage B: overlap gated pooling -> pooledT / kT / V ----
        with (
            tc.tile_pool(name="poolb", bufs=1) as pb,
            tc.tile_pool(name="psB", bufs=4, space="PSUM") as psB,
        ):
            numer = pb.tile([HD, NW], F32, tag="numer", name="numer")
            denom = pb.tile([HD, NW], F32, tag="denom", name="denom")
            for half, (g_src, kv_src, acol) in enumerate(
                ((glo, kvlo, slice(0, RATIO)),
                 (ghi, kvhi, slice(RATIO, 2 * RATIO)))
            ):
                n = (RATIO + S) if half == 0 else S
                e = pb.tile([HD, RATIO + S], F32, tag="poole", name="poole")
                nc.scalar.activation(e[:, 0:n], g_src[:, 0:n], AF.Exp)
                nc.vector.tensor_mul(
                    e[:, 0:n].rearrange("d (w r) -> d w r", r=RATIO),
                    e[:, 0:n].rearrange("d (w r) -> d w r", r=RATIO),
                    eape_s[:, None, acol].to_broadcast([HD, n // RATIO, RATIO]))
                ea = pb.tile([HD, RATIO + S], F32, tag="poolea", name="poolea")
                nc.vector.tensor_mul(ea[:, 0:n], e[:, 0:n], kv_src[:, 0:n])
                # windowed sums over the 4 slots via strided adds
                for acc, src in ((denom, e), (numer, ea)):
                    s3 = src[:, 0:S].rearrange("d (w r) -> d w r", r=RATIO)
                    nm = f"poolred{half}{1 if acc is numer else 0}"
                    ra = pb.tile([HD, NW], F32, tag="poolra", name=nm + "a")
                    nc.vector.tensor_add(ra[:], s3[:, :, 0], s3[:, :, 1])
                    rc = pb.tile([HD, NW], F32, tag="poolrc", name=nm + "c")
                    nc.vector.tensor_add(rc[:], s3[:, :, 2], s3[:, :, 3])
                    if half == 0:
                        nc.vector.tensor_add(acc[:], ra[:], rc[:])
                    else:
                        nc.vector.tensor_add(ra[:], ra[:], rc[:])
                        nc.vector.tensor_add(acc[:], acc[:], ra[:])
            rec = pb.tile([HD, NW], F32, tag="poolrec", name="poolrec")
            nc.vector.reciprocal(rec[:], denom[:])
            nc.vector.tensor_mul(pooledT[:], numer[:], rec[:])

            # V = pooledT^T (PE transpose, 128x128 blocks), bf16
            for wb in range(NW // WCH):
                tp = psB.tile([128, 128], F32, tag="vtrans", name="vtrans")
                nc.tensor.transpose(tp[:], pooledT[:, ts(wb, 128)], ident[:])
                nc.scalar.copy(v_s[:, wb, :], tp[:])

            # kT = partial interleaved RoPE of pooledT at positions w*RATIO
            nc.scalar.copy(kT[0:RD, :], pooledT[0:RD, :])
            krb = pb.tile([128, NW], BF16, tag="krope", name="krope")
            nc.scalar.copy(krb[RD:128, :], pooledT[RD:128, :])
            ksw = pb.tile([128, NW], BF16, tag="kswap", name="kswap")
            nc.vector.stream_shuffle(ksw[RD:128, :], krb[RD:128, :], _PAIR_SWAP)
            kt1 = pb.tile([128, NW], BF16, tag="kt1", name="kt1")
            nc.vector.tensor_mul(kt1[RD:128, :], krb[RD:128, :],
                                 cosk_s[RD:128, :])
            kt2 = pb.tile([128, NW], BF16, tag="kt2", name="kt2")
            nc.vector.tensor_mul(kt2[RD:128, :], ksw[RD:128, :],
                                 sink_s[RD:128, :])
            nc.vector.tensor_add(kT[RD:128, :], kt1[RD:128, :], kt2[RD:128, :])

        # ---- stage C: attention + output projection, per q-chunk ----
        with (
            tc.tile_pool(name="pt", bufs=4) as ptp,
            tc.tile_pool(name="att", bufs=2) as att,
            tc.tile_pool(name="psS", bufs=3, space="PSUM") as psS,
            tc.tile_pool(name="psRB", bufs=1, space="PSUM") as psrb,
            tc.tile_pool(name="psO", bufs=2, space="PSUM") as pso,
            tc.tile_pool(name="psAcc", bufs=1, space="PSUM") as psacc,
        ):
            for j in range(NCH):
                tsl = ts(j, TCH)
                attnT = []
                for h in range(HPC):
                    qsl = qT[h][:, tsl]
                    den_ps = psacc.tile([1, TCH], F32, tag="den", name="den")
                    acc_ps = psacc.tile([128, TCH], F32, tag="acc", name="acc")
                    # score matmuls run one w-chunk ahead of the exp/sum/PV
                    # consumers so PE never sits behind the ACT exp
                    s_tiles = {}

                    def _score(wc, s_tiles=s_tiles, qsl=qsl):
                        sp = psS.tile([WCH, TCH], F32, tag="s", name="s")
                        nc.tensor.matmul(sp[:], kT[:, ts(wc, WCH)], qsl,
                                         start=True, stop=True)
                        s_tiles[wc] = sp

                    _score(0)
                    for wc in range(j + 1):
                        if wc < j:
                            _score(wc + 1)
                        s_ps = s_tiles.pop(wc)
                        pt = ptp.tile([WCH, TCH], BF16, tag="pt", name="pt")
                        nc.scalar.activation(pt[:], s_ps[:], AF.Exp,
                                             scale=SCALE)
                        if wc == j:
                            ptm = ptp.tile([WCH, TCH], BF16, tag="ptm", name="ptm")
                            nc.vector.tensor_mul(ptm[:], pt[:], bandm_s[:])
                            pt = ptm
                        nc.tensor.matmul(den_ps[:], ones_w[:], pt[:],
                                         start=(wc == 0), stop=(wc == j))
                        nc.tensor.matmul(acc_ps[:], v_s[:, wc, :], pt[:],
                                         start=(wc == 0), stop=(wc == j))
                    den_sb = att.tile([1, TCH], F32, tag="den_sb", name="den_sb")
                    nc.scalar.activation(den_sb[:], den_ps[:], AF.Identity,
                                         bias=esink_s[0:1, h:h + 1])
                    rec_sb = att.tile([1, TCH], F32, tag="rec_sb", name="rec_sb")
                    nc.vector.reciprocal(rec_sb[:], den_sb[:])
                    rb_ps = psrb.tile([128, TCH], F32, tag="rb", name="rb")
                    nc.tensor.matmul(rb_ps[:], ones_p[:], rec_sb[:],
                                     start=True, stop=True)
                    rb_sb = att.tile([128, TCH], F32, tag="rb_sb", name="rb_sb")
                    nc.scalar.copy(rb_sb[:], rb_ps[:])
                    a_sb = att.tile([128, TCH], BF16, tag=f"attnT{h}", name=f"attnT{h}")
                    nc.vector.tensor_mul(a_sb[:], acc_ps[:], rb_sb[:])
                    attnT.append(a_sb)

                for tt in range(TCH // 128):
                    for e in range(HID // TCH):
                        o_ps = pso.tile([128, TCH], F32, tag="o", name="o")
                        for h in range(HPC):
                            nc.tensor.matmul(o_ps[:], attnT[h][:, ts(tt, 128)],
                                             wo_s[:, h, ts(e, TCH)],
                                             start=(h == 0), stop=(h == HPC - 1))
                        o_sb = att.tile([128, TCH], F32, tag="o_sb", name="o_sb")
                        if e % 2 == 0:
                            nc.scalar.copy(o_sb[:], o_ps[:])
                        else:
                            nc.vector.tensor_copy(o_sb[:], o_ps[:])
                        nc.sync.dma_start(
                            out[j * TCH + tt * 128:j * TCH + (tt + 1) * 128,
                                ts(e, TCH)], o_sb[:])


_WS_CTR = [0]


def _split_multi_waits(nc):
    """This walrus build accepts at most ONE sync wait per instruction; hoist
    extras onto same-engine NOPs placed immediately before."""
    f = nc.m.functions[0]
    for blk in f.blocks:
        insts = blk.instructions
        if not any(i.sync_info is not None and len(i.sync_info.on_wait) > 1
                   for i in insts):
            continue
        new_list = []
        for inst in insts:
            si = inst.sync_info
            if si is not None and len(si.on_wait) > 1:
                waits = list(si.on_wait)
                for w in waits[:-1]:
                    _WS_CTR[0] += 1
                    new_list.append(mybir.InstNoOp(
                        name=f"waitsplit-{_WS_CTR[0]}",
                        engine=inst.engine,
                        bass_nofuse=True,
                        sync_info=mybir.SyncInfo(on_wait=[w], on_update=[])))
                inst.sync_info = mybir.SyncInfo(
                    on_wait=[waits[-1]], on_update=list(si.on_update))
            new_list.append(inst)
        blk.instructions = new_list


# ---------------------------------------------------------------------------
# host side
# ---------------------------------------------------------------------------

def _rope_tables(positions):
    half = RD // 2
    inv_freq = 1.0 / (THETA ** (np.arange(half, dtype=np.float64) / half))
    ang = positions[None, :].astype(np.float64) * inv_freq[:, None]  # [32, L]
    cos_t = np.repeat(np.cos(ang), 2, axis=0).astype(np.float32)
    sin_t = np.repeat(np.sin(ang), 2, axis=0).astype(np.float32)
    sin_t[0::2] *= -1.0                                  # a-rows get -sin
    return cos_t, sin_t


def _prep_inputs(hidden, wq, wkv, wgate, ape, sinks, wo):
    bf = ml_dtypes.bfloat16
    cosq_t, sinq_t = _rope_tables(np.arange(S))
    cosk_t, sink_t = _rope_tables(np.arange(NW) * RATIO)
    pw, ft = np.meshgrid(np.arange(WCH), np.arange(TCH), indexing="ij")
    band = (ft >= RATIO * pw + RATIO - 1).astype(np.float32)     # [WCH, TCH]
    eape = np.empty((HD, 2 * RATIO), np.float32)
    for r in range(RATIO):
        eape[:, r] = np.exp(ape[r, :HD])
        eape[:, RATIO + r] = np.exp(ape[r, HD:])
    maps = []
    for c in range(N_CORES):
        b, g = divmod(c, HPC)
        maps.append({
            "ht": np.ascontiguousarray(hidden[b].T).astype(bf),
            "wq": np.ascontiguousarray(wq[:, g * CW:(g + 1) * CW]).astype(bf),
            "wkv": wkv.astype(bf),
            "wg": wgate.astype(bf),
            "wo": np.ascontiguousarray(wo[g * CW:(g + 1) * CW, :]).astype(bf),
            "eape": eape,
            "esink": np.exp(sinks[g * HPC:(g + 1) * HPC]).astype(
                np.float32).reshape(1, HPC),
            "cosq": cosq_t.astype(bf), "sinq": sinq_t.astype(bf),
            "cosk": cosk_t.astype(bf), "sink": sink_t.astype(bf),
            "bandm": band.astype(bf),
        })
    return maps


_RUNNER_CACHE = {}


def _get_runner(n_reps: int = 1):
    if n_reps in _RUNNER_CACHE:
        return _RUNNER_CACHE[n_reps]
    import jax
    from jax.sharding import Mesh, PartitionSpec
    from jax.experimental.shard_map import shard_map
    from concourse.bass2jax import (_bass_exec_p, install_neuronx_cc_hook,
                                    partition_id_tensor)

    nc = _build_nc(n_reps)
    install_neuronx_cc_hook()
    partition_name = nc.partition_id_tensor.name if nc.partition_id_tensor else None
    in_names, out_names, out_avals, zero_outs = [], [], [], []
    for alloc in nc.m.functions[0].allocations:
        if not isinstance(alloc, mybir.MemoryLocationSet):
            continue
        name = alloc.memorylocations[0].name
        if alloc.kind == "ExternalInput":
            if name != partition_name:
                in_names.append(name)
        elif alloc.kind == "ExternalOutput":
            out_names.append(name)
            shape = tuple(alloc.tensor_shape)
            dtype = mybir.dt.np(alloc.dtype)
            out_avals.append(jax.core.ShapedArray(shape, dtype))
            zero_outs.append(np.zeros(shape, dtype))
    n_params = len(in_names)
    all_in_names = list(in_names) + out_names
    if partition_name is not None:
        all_in_names.append(partition_name)

    def _kernel_body(*args):
        operands = list(args)
        if partition_name is not None:
            operands.append(partition_id_tensor())
        outs = _bass_exec_p.bind(
            *operands,
            out_avals=tuple(out_avals),
            in_names=tuple(all_in_names),
            out_names=tuple(out_names),
            lowering_input_output_aliases=(),
            sim_require_finite=True,
            sim_require_nnan=True,
            nc=nc,
        )
        return tuple(outs)

    devices = jax.devices()[:N_CORES]
    mesh = Mesh(np.asarray(devices), ("core",))
    spec = PartitionSpec("core")
    fn = jax.jit(shard_map(
        _kernel_body, mesh=mesh,
        in_specs=(spec,) * (n_params + len(out_names)),
        out_specs=(spec,) * len(out_names), check_rep=False))
    runner = (fn, in_names, out_names, zero_outs, mesh)
    _RUNNER_CACHE[n_reps] = runner
    return runner


def _run_core_maps(maps, n_reps: int = 1):
    import jax
    from jax.sharding import NamedSharding, PartitionSpec
    fn, in_names, out_names, zero_outs, mesh = _get_runner(n_reps)
    sh = NamedSharding(mesh, PartitionSpec("core"))
    args = [jax.device_put(
        np.concatenate([np.asarray(m[name]) for m in maps], axis=0), sh)
        for name in in_names]
    for z in zero_outs:
        args.append(jax.device_put(
            np.zeros((N_CORES * z.shape[0], *z.shape[1:]), z.dtype), sh))
    res = fn(*args)
    jax.block_until_ready(res)
    return np.asarray(res[0]).reshape(N_CORES, S, HID)


def kernel(hidden, wq, wkv, wgate, ape, sinks, wo,
           ratio=RATIO, head_dim=HD, rope_head_dim=RD, num_heads=NH):
    hidden = np.asarray(hidden, np.float32)
    maps = _prep_inputs(hidden, np.asarray(wq, np.float32),
                        np.asarray(wkv, np.float32),
                        np.asarray(wgate, np.float32),
                        np.asarray(ape, np.float32),
                        np.asarray(sinks, np.float32),
                        np.asarray(wo, np.float32))
    partials = _run_core_maps(maps)
    out = np.empty((B, S, HID), np.float32)
    for b in range(B):
        out[b] = partials[b * HPC:(b + 1) * HPC].astype(np.float64).sum(
            axis=0).astype(np.float32)
    return out



# revision 2
# speedup vs baseline: 2.0549x; 2.0549x over previous
"""DeepseekV4-style sparse attention on 8 Trainium2 cores (Bass/Tile), v2.

Sharding: data-parallel over batch (2) x tensor-parallel over heads (16 -> 4
groups of 4).  Core c handles batch c//4 and heads [4*(c%4), 4*(c%4)+4).
Per-core partial outputs (attn_heads @ wo_rows) are summed on the host.

v2 structure (vs v1): fully software-pipelined per 512-token chunk so the PE
engine never idles behind the pooling chain:
    for j: emit B_{j-1} (pooling, ACT/DVE) ; A_j (projections, PE) ;
           C_{j-1} (attention + out-proj, PE)
  - softmax denominator is computed as ones[128,128]^T @ exp(S^T), giving the
    denominator replicated across all 128 partitions ("rbden") -- no separate
    broadcast matmul / copy; sink is added via DVE tensor_scalar, then
    reciprocal, then one DVE multiply normalizes PV.
  - kv/gate slabs for pooling live in 2-chunk rings with a RATIO-column halo
    (lo halves only -- the hi halves pool the current window).
  - initial DMAs are split per-k-group and ordered so the first matmul can
    start after ~1/4 of wq + hslab0 arrived.
"""

import numpy as np
import ml_dtypes

import concourse.bass as bass
import concourse.mybir as mybir
import concourse.tile as tile
from concourse.bass import ts
from concourse.masks import make_identity

F32 = mybir.dt.float32
BF16 = mybir.dt.bfloat16
AF = mybir.ActivationFunctionType

B, S, HID, NH, HD, RD, RATIO = 2, 4096, 2048, 16, 128, 64, 4
THETA = 10000.0
NW = S // RATIO
N_CORES = 8
HPC = 4
CW = HPC * HD
TCH = 512
NCH = S // TCH
WCH = 128
KCH = HID // 128
SCALE = HD ** -0.5

_PAIR_SWAP = [i ^ 1 for i in range(32)]


def _build_nc(n_reps: int = 1, split_waits: bool = True):
    nc = bass.Bass()
    dp = nc.declare_dram_parameter
    ht = dp("ht", [HID, S], BF16, isOutput=False)
    wq = dp("wq", [HID, CW], BF16, isOutput=False)
    wkv = dp("wkv", [HID, 2 * HD], BF16, isOutput=False)
    wg = dp("wg", [HID, 2 * HD], BF16, isOutput=False)
    wo = dp("wo", [CW, HID], BF16, isOutput=False)
    eape = dp("eape", [HD, 2 * RATIO], F32, isOutput=False)
    esinkb = dp("esinkb", [128, HPC], F32, isOutput=False)
    cosq = dp("cosq", [RD, S], BF16, isOutput=False)
    sinq = dp("sinq", [RD, S], BF16, isOutput=False)
    cosk = dp("cosk", [RD, NW], BF16, isOutput=False)
    sink = dp("sink", [RD, NW], BF16, isOutput=False)
    bandm = dp("bandm", [WCH, TCH], BF16, isOutput=False)
    out = dp("out", [S, HID], F32, isOutput=True)
    args = (ht, wq, wkv, wg, wo, eape, esinkb, cosq, sinq, cosk, sink, bandm,
            out)

    with tile.TileContext(nc) as tc:
        if n_reps > 1:
            with tc.For_i(0, n_reps, 1):
                _body(nc, tc, *args)
        else:
            _body(nc, tc, *args)
    if split_waits:
        _split_multi_waits(nc)
    return nc


def _body(nc, tc, ht, wq, wkv, wg, wo, eape, esinkb,
          cosq, sinq, cosk, sink, bandm, out):
    HLO = RATIO + TCH               # lo-half ring slab width (4-col halo)
    with (
        tc.tile_pool(name="persist", bufs=1) as pp,
        tc.tile_pool(name="wts", bufs=1) as wts,
        tc.tile_pool(name="hslab", bufs=2) as hs,
        tc.tile_pool(name="ev", bufs=3) as ev,
        tc.tile_pool(name="bwork", bufs=1) as bw,
        tc.tile_pool(name="att", bufs=2) as att,
        tc.tile_pool(name="psAO", bufs=2, space="PSUM") as psAO,
        tc.tile_pool(name="psS", bufs=2, space="PSUM") as psS,
        tc.tile_pool(name="psAcc", bufs=2, space="PSUM") as psAcc,
        tc.tile_pool(name="psRB", bufs=2, space="PSUM") as psRB,
    ):
        # ---- persistent SBUF state ----
        qT = [pp.tile([128, S], BF16, tag=f"qT{m}", name=f"qT{m}")
              for m in range(HPC)]
        # kv/gate rings (2 chunk slots); lo halves carry a RATIO-col halo
        kvlo = pp.tile([128, 2, HLO], BF16, tag="kvlo", name="kvlo")
        glo = pp.tile([128, 2, HLO], BF16, tag="glo", name="glo")
        kvhi = pp.tile([128, 2, TCH], BF16, tag="kvhi", name="kvhi")
        ghi = pp.tile([128, 2, TCH], BF16, tag="ghi", name="ghi")
        cosq_s = pp.tile([128, S], BF16, tag="cosq", name="cosq")
        sinq_s = pp.tile([128, S], BF16, tag="sinq", name="sinq")
        cosk_s = pp.tile([128, NW], BF16, tag="cosk", name="cosk")
        sink_s = pp.tile([128, NW], BF16, tag="sink", name="sink")
        eape_s = pp.tile([HD, 2 * RATIO], F32, tag="eape", name="eape")
        esink_s = pp.tile([128, HPC], F32, tag="esink", name="esink")
        bandm_s = pp.tile([WCH, TCH], BF16, tag="bandm", name="bandm")
        wo_s = pp.tile([HD, HPC, HID], BF16, tag="wo", name="wo")
        ones_w = pp.tile([WCH, WCH], BF16, tag="ones_w", name="ones_w")
        kT = pp.tile([HD, NW], BF16, tag="kT", name="kT")
        v_s = pp.tile([WCH, NW // WCH, HD], BF16, tag="v", name="v")
        ident = pp.tile([128, 128], F32, tag="ident", name="ident")

        wq_s = wts.tile([128, KCH, CW], BF16, tag="wq", name="wq")
        wkv_s = wts.tile([128, KCH, 2 * HD], BF16, tag="wkv", name="wkv")
        wg_s = wts.tile([128, KCH, 2 * HD], BF16, tag="wg", name="wg")

        # ---- preamble: ordered DMAs (chunk-0-critical first) ----
        htr = ht.rearrange("(k p) t -> p k t", p=128)
        wqr = wq.rearrange("(k p) c -> p k c", p=128)
        hsl0 = hs.tile([128, KCH, TCH], BF16, tag="hslab", name="hslab0")
        for g in range(4):
            ksl = slice(4 * g, 4 * g + 4)
            nc.sync.dma_start(wq_s[:, ksl, :], wqr[:, ksl, :])
            nc.sync.dma_start(hsl0[:, ksl, :], htr[:, ksl, ts(0, TCH)])
        nc.sync.dma_start(wkv_s[:], wkv.rearrange("(k p) c -> p k c", p=128))
        nc.sync.dma_start(wg_s[:], wg.rearrange("(k p) c -> p k c", p=128))
        nc.sync.dma_start(cosq_s[RD:128, :], cosq[:])
        nc.sync.dma_start(sinq_s[RD:128, :], sinq[:])
        nc.sync.dma_start(eape_s[:], eape[:])
        nc.sync.dma_start(esink_s[:], esinkb[:])
        nc.sync.dma_start(bandm_s[:], bandm[:])
        nc.sync.dma_start(cosk_s[RD:128, :], cosk[:])
        nc.sync.dma_start(sink_s[RD:128, :], sink[:])
        nc.sync.dma_start(wo_s[:], wo.rearrange("(h p) e -> p h e", p=HD))
        nc.vector.memset(ones_w[:], 1.0)
        nc.gpsimd.memset(kvlo[:, 0, 0:RATIO], 0.0)
        nc.gpsimd.memset(glo[:, 0, 0:RATIO], -30000.0)
        make_identity(nc, ident[:])

        hsl_tiles = {0: hsl0}

        def stage_a(j):
            tsl = ts(j, TCH)
            hsl = hsl_tiles.pop(j)
            # prefetch next chunk's hidden slab
            if j + 1 < NCH:
                nxt = hs.tile([128, KCH, TCH], BF16, tag="hslab",
                              name=f"hslab{j + 1}")
                for g in range(4):
                    ksl = slice(4 * g, 4 * g + 4)
                    nc.sync.dma_start(nxt[:, ksl, :],
                                      htr[:, ksl, ts(j + 1, TCH)])
                hsl_tiles[j + 1] = nxt

            # q projection with fused partial interleaved RoPE
            for m in range(HPC):
                ps = psAO.tile([128, TCH], F32, tag="a", name="aq")
                for k in range(KCH):
                    nc.tensor.matmul(ps[:], wq_s[:, k, ts(m, 128)],
                                     hsl[:, k, :], start=(k == 0),
                                     stop=(k == KCH - 1))
                nc.scalar.copy(qT[m][0:RD, tsl], ps[0:RD, :])
                rb = ev.tile([128, TCH], BF16, tag="ropebuf", name="ropebuf")
                nc.scalar.copy(rb[RD:128, :], ps[RD:128, :])
                sw = ev.tile([128, TCH], BF16, tag="ropeswap", name="ropeswap")
                nc.vector.stream_shuffle(sw[RD:128, :], rb[RD:128, :],
                                         _PAIR_SWAP)
                t1 = ev.tile([128, TCH], BF16, tag="ropet1", name="ropet1")
                nc.vector.tensor_mul(t1[RD:128, :], rb[RD:128, :],
                                     cosq_s[RD:128, tsl])
                t2 = ev.tile([128, TCH], BF16, tag="ropet2", name="ropet2")
                nc.vector.tensor_mul(t2[RD:128, :], sw[RD:128, :],
                                     sinq_s[RD:128, tsl])
                nc.vector.tensor_add(qT[m][RD:128, tsl], t1[RD:128, :],
                                     t2[RD:128, :])

            # kv / gate projections into the 2-slot rings
            sj = j % 2
            sj1 = (j + 1) % 2
            for dst, lo, wsrc in (
                (kvlo, True, wkv_s), (kvhi, False, wkv_s),
                (glo, True, wg_s), (ghi, False, wg_s),
            ):
                ps = psAO.tile([128, TCH], F32, tag="a", name="akv")
                col = slice(0, 128) if lo else slice(128, 256)
                for k in range(KCH):
                    nc.tensor.matmul(ps[:], wsrc[:, k, col], hsl[:, k, :],
                                     start=(k == 0), stop=(k == KCH - 1))
                if lo:
                    nc.scalar.copy(dst[:, sj, RATIO:HLO], ps[:])
                    if j + 1 < NCH:   # halo for the next chunk's windows
                        nc.scalar.copy(dst[:, sj1, 0:RATIO],
                                       ps[:, TCH - RATIO:TCH])
                else:
                    nc.scalar.copy(dst[:, sj, :], ps[:])

        def stage_b(j):
            """Pool chunk j's windows -> kT[:, j*WCH:...], v_s[:, j, :]."""
            sj = j % 2
            wsl = ts(j, WCH)
            numer = bw.tile([HD, WCH], F32, tag="numer", name="numer")
            denom = bw.tile([HD, WCH], F32, tag="denom", name="denom")
            for half, (g_src, kv_src, acol, width, off) in enumerate((
                (glo[:, sj, :], kvlo[:, sj, :], slice(0, RATIO), HLO, 0),
                (ghi[:, sj, :], kvhi[:, sj, :], slice(RATIO, 2 * RATIO),
                 TCH, 0),
            )):
                e = bw.tile([HD, HLO], F32, tag="poole", name=f"poole{half}")
                nc.scalar.activation(e[:, 0:width], g_src[:, 0:width], AF.Exp)
                nc.vector.tensor_mul(
                    e[:, 0:width].rearrange("d (w r) -> d w r", r=RATIO),
                    e[:, 0:width].rearrange("d (w r) -> d w r", r=RATIO),
                    eape_s[:, None, acol].to_broadcast(
                        [HD, width // RATIO, RATIO]))
                ea = bw.tile([HD, HLO], F32, tag="poolea", name=f"poolea{half}")
                nc.vector.tensor_mul(ea[:, 0:width], e[:, 0:width],
                                     kv_src[:, 0:width])
                for acc, src in ((denom, e), (numer, ea)):
                    s3 = src[:, off:off + TCH].rearrange(
                        "d (w r) -> d w r", r=RATIO)
                    nm = f"poolred{half}{1 if acc is numer else 0}"
                    ra = bw.tile([HD, WCH], F32, tag="poolra", name=nm + "a")
                    nc.vector.tensor_add(ra[:], s3[:, :, 0], s3[:, :, 1])
                    rc = bw.tile([HD, WCH], F32, tag="poolrc", name=nm + "c")
                    nc.vector.tensor_add(rc[:], s3[:, :, 2], s3[:, :, 3])
                    if half == 0:
                        nc.vector.tensor_add(acc[:], ra[:], rc[:])
                    else:
                        nc.vector.tensor_add(ra[:], ra[:], rc[:])
                        nc.vector.tensor_add(acc[:], acc[:], ra[:])
            rec = bw.tile([HD, WCH], F32, tag="poolrec", name="poolrec")
            nc.vector.reciprocal(rec[:], denom[:])
            pooledc = bw.tile([HD, WCH], F32, tag="pooledc", name="pooledc")
            nc.vector.tensor_mul(pooledc[:], numer[:], rec[:])

            # V chunk = pooledc^T via PE transpose (shares the psAO pool)
            tp = psAO.tile([128, TCH], F32, tag="a", name="vtrans")
            nc.tensor.transpose(tp[:, 0:WCH], pooledc[:], ident[:])
            nc.scalar.copy(v_s[:, j, :], tp[:, 0:WCH])

            # kT chunk = partial interleaved RoPE at positions w*RATIO
            nc.scalar.copy(kT[0:RD, wsl], pooledc[0:RD, :])
            krb = bw.tile([128, WCH], BF16, tag="krope", name="krope")
            nc.scalar.copy(krb[RD:128, :], pooledc[RD:128, :])
            ksw = bw.tile([128, WCH], BF16, tag="kswap", name="kswap")
            nc.vector.stream_shuffle(ksw[RD:128, :], krb[RD:128, :],
                                     _PAIR_SWAP)
            kt1 = bw.tile([128, WCH], BF16, tag="kt1", name="kt1")
            nc.vector.tensor_mul(kt1[RD:128, :], krb[RD:128, :],
                                 cosk_s[RD:128, wsl])
            kt2 = bw.tile([128, WCH], BF16, tag="kt2", name="kt2")
            nc.vector.tensor_mul(kt2[RD:128, :], ksw[RD:128, :],
                                 sink_s[RD:128, wsl])
            nc.vector.tensor_add(kT[RD:128, wsl], kt1[RD:128, :],
                                 kt2[RD:128, :])

        def stage_c(jc):
            """Attention + output projection for query chunk jc."""
            tsl = ts(jc, TCH)
            a_sb = []
            for h in range(HPC):
                qsl = qT[h][:, tsl]
                rbden = psRB.tile([128, TCH], F32, tag="rb", name="rbden")
                acc_ps = psAcc.tile([128, TCH], F32, tag="acc", name="acc")
                s_tiles = {}

                def _score(wc, s_tiles=s_tiles, qsl=qsl):
                    sp = psS.tile([WCH, TCH], F32, tag="s", name="s")
                    nc.tensor.matmul(sp[:], kT[:, ts(wc, WCH)], qsl,
                                     start=True, stop=True)
                    s_tiles[wc] = sp

                _score(0)
                for wc in range(jc + 1):
                    if wc < jc:
                        _score(wc + 1)
                    s_ps = s_tiles.pop(wc)
                    pt = ev.tile([WCH, TCH], BF16, tag="pt", name="pt")
                    nc.scalar.activation(pt[:], s_ps[:], AF.Exp, scale=SCALE)
                    if wc == jc:
                        ptm = ev.tile([WCH, TCH], BF16, tag="ptm", name="ptm")
                        nc.vector.tensor_mul(ptm[:], pt[:], bandm_s[:])
                        pt = ptm
                    nc.tensor.matmul(rbden[:], ones_w[:], pt[:],
                                     start=(wc == 0), stop=(wc == jc))
                    nc.tensor.matmul(acc_ps[:], v_s[:, wc, :], pt[:],
                                     start=(wc == 0), stop=(wc == jc))
                dsink = att.tile([128, TCH], F32, tag="dsink", name="dsink")
                nc.vector.tensor_scalar_add(dsink[:], rbden[:],
                                            esink_s[:, h:h + 1])
                rec_sb = att.tile([128, TCH], F32, tag="rec_sb", name="rec_sb")
                nc.vector.reciprocal(rec_sb[:], dsink[:])
                a = att.tile([128, TCH], BF16, tag=f"attnT{h}",
                             name=f"attnT{h}")
                nc.vector.tensor_mul(a[:], acc_ps[:], rec_sb[:])
                a_sb.append(a)

            for tt in range(TCH // 128):
                for e in range(HID // TCH):
                    o_ps = psAO.tile([128, TCH], F32, tag="a", name="o")
                    for h in range(HPC):
                        nc.tensor.matmul(o_ps[:], a_sb[h][:, ts(tt, 128)],
                                         wo_s[:, h, ts(e, TCH)],
                                         start=(h == 0), stop=(h == HPC - 1))
                    o_sb = att.tile([128, TCH], F32, tag="o_sb", name="o_sb",
                                    bufs=3)
                    if e % 2 == 0:
                        nc.scalar.copy(o_sb[:], o_ps[:])
                    else:
                        nc.vector.tensor_copy(o_sb[:], o_ps[:])
                    nc.sync.dma_start(
                        out[jc * TCH + tt * 128:jc * TCH + (tt + 1) * 128,
                            ts(e, TCH)], o_sb[:])

        for j in range(NCH):
            if j > 0:
                stage_b(j - 1)
            stage_a(j)
            if j > 0:
                stage_c(j - 1)
        stage_b(NCH - 1)
        stage_c(NCH - 1)


_WS_CTR = [0]


def _split_multi_waits(nc):
    """This walrus build accepts at most ONE sync wait per instruction; hoist
    extras onto same-engine NOPs placed immediately before."""
    f = nc.m.functions[0]
    for blk in f.blocks:
        insts = blk.instructions
        if not any(i.sync_info is not None and len(i.sync_info.on_wait) > 1
                   for i in insts):
            continue
        new_list = []
        for inst in insts:
            si = inst.sync_info
            if si is not None and len(si.on_wait) > 1:
                waits = list(si.on_wait)
                for w in waits[:-1]:
                    _WS_CTR[0] += 1
                    new_list.append(mybir.InstNoOp(
                        name=f"waitsplit-{_WS_CTR[0]}",
                        engine=inst.engine,
                        bass_nofuse=True,
                        sync_info=mybir.SyncInfo(on_wait=[w], on_update=[])))
                inst.sync_info = mybir.SyncInfo(
                    on_wait=[waits[-1]], on_update=list(si.on_update))
            new_list.append(inst)
        blk.instructions = new_list


# ---------------------------------------------------------------------------
# host side
# ---------------------------------------------------------------------------

def _rope_tables(positions):
    half = RD // 2
    inv_freq = 1.0 / (THETA ** (np.arange(half, dtype=np.float64) / half))
    ang = positions[None, :].astype(np.float64) * inv_freq[:, None]  # [32, L]
    cos_t = np.repeat(np.cos(ang), 2, axis=0).astype(np.float32)
    sin_t = np.repeat(np.sin(ang), 2, axis=0).astype(np.float32)
    sin_t[0::2] *= -1.0                                  # a-rows get -sin
    return cos_t, sin_t


def _prep_inputs(hidden, wq, wkv, wgate, ape, sinks, wo):
    bf = ml_dtypes.bfloat16
    cosq_t, sinq_t = _rope_tables(np.arange(S))
    cosk_t, sink_t = _rope_tables(np.arange(NW) * RATIO)
    pw, ft = np.meshgrid(np.arange(WCH), np.arange(TCH), indexing="ij")
    band = (ft >= RATIO * pw + RATIO - 1).astype(np.float32)     # [WCH, TCH]
    eape = np.empty((HD, 2 * RATIO), np.float32)
    for r in range(RATIO):
        eape[:, r] = np.exp(ape[r, :HD])
        eape[:, RATIO + r] = np.exp(ape[r, HD:])
    maps = []
    for c in range(N_CORES):
        b, g = divmod(c, HPC)
        esink = np.exp(sinks[g * HPC:(g + 1) * HPC]).astype(np.float32)
        maps.append({
            "ht": np.ascontiguousarray(hidden[b].T).astype(bf),
            "wq": np.ascontiguousarray(wq[:, g * CW:(g + 1) * CW]).astype(bf),
            "wkv": wkv.astype(bf),
            "wg": wgate.astype(bf),
            "wo": np.ascontiguousarray(wo[g * CW:(g + 1) * CW, :]).astype(bf),
            "eape": eape,
            "esinkb": np.broadcast_to(esink[None, :], (128, HPC)).copy(),
            "cosq": cosq_t.astype(bf), "sinq": sinq_t.astype(bf),
            "cosk": cosk_t.astype(bf), "sink": sink_t.astype(bf),
            "bandm": band.astype(bf),
        })
    return maps


_RUNNER_CACHE = {}


def _get_runner(n_reps: int = 1):
    if n_reps in _RUNNER_CACHE:
        return _RUNNER_CACHE[n_reps]
    import jax
    from jax.sharding import Mesh, PartitionSpec
    from jax.experimental.shard_map import shard_map
    from concourse.bass2jax import (_bass_exec_p, install_neuronx_cc_hook,
                                    partition_id_tensor)

    nc = _build_nc(n_reps)
    install_neuronx_cc_hook()
    partition_name = nc.partition_id_tensor.name if nc.partition_id_tensor else None
    in_names, out_names, out_avals, zero_outs = [], [], [], []
    for alloc in nc.m.functions[0].allocations:
        if not isinstance(alloc, mybir.MemoryLocationSet):
            continue
        name = alloc.memorylocations[0].name
        if alloc.kind == "ExternalInput":
            if name != partition_name:
                in_names.append(name)
        elif alloc.kind == "ExternalOutput":
            out_names.append(name)
            shape = tuple(alloc.tensor_shape)
            dtype = mybir.dt.np(alloc.dtype)
            out_avals.append(jax.core.ShapedArray(shape, dtype))
            zero_outs.append(np.zeros(shape, dtype))
    n_params = len(in_names)
    all_in_names = list(in_names) + out_names
    if partition_name is not None:
        all_in_names.append(partition_name)

    def _kernel_body(*args):
        operands = list(args)
        if partition_name is not None:
            operands.append(partition_id_tensor())
        outs = _bass_exec_p.bind(
            *operands,
            out_avals=tuple(out_avals),
            in_names=tuple(all_in_names),
            out_names=tuple(out_names),
            lowering_input_output_aliases=(),
            sim_require_finite=True,
            sim_require_nnan=True,
            nc=nc,
        )
        return tuple(outs)

    devices = jax.devices()[:N_CORES]
    mesh = Mesh(np.asarray(devices), ("core",))
    spec = PartitionSpec("core")
    fn = jax.jit(shard_map(
        _kernel_body, mesh=mesh,
        in_specs=(spec,) * (n_params + len(out_names)),
        out_specs=(spec,) * len(out_names), check_rep=False))
    runner = (fn, in_names, out_names, zero_outs, mesh)
    _RUNNER_CACHE[n_reps] = runner
    return runner


def _run_core_maps(maps, n_reps: int = 1):
    import jax
    from jax.sharding import NamedSharding, PartitionSpec
    fn, in_names, out_names, zero_outs, mesh = _get_runner(n_reps)
    sh = NamedSharding(mesh, PartitionSpec("core"))
    args = [jax.device_put(
        np.concatenate([np.asarray(m[name]) for m in maps], axis=0), sh)
        for name in in_names]
    for z in zero_outs:
        args.append(jax.device_put(
            np.zeros((N_CORES * z.shape[0], *z.shape[1:]), z.dtype), sh))
    res = fn(*args)
    jax.block_until_ready(res)
    return np.asarray(res[0]).reshape(N_CORES, S, HID)


def kernel(hidden, wq, wkv, wgate, ape, sinks, wo,
           ratio=RATIO, head_dim=HD, rope_head_dim=RD, num_heads=NH):
    hidden = np.asarray(hidden, np.float32)
    maps = _prep_inputs(hidden, np.asarray(wq, np.float32),
                        np.asarray(wkv, np.float32),
                        np.asarray(wgate, np.float32),
                        np.asarray(ape, np.float32),
                        np.asarray(sinks, np.float32),
                        np.asarray(wo, np.float32))
    partials = _run_core_maps(maps)
    out = np.empty((B, S, HID), np.float32)
    for b in range(B):
        out[b] = partials[b * HPC:(b + 1) * HPC].astype(np.float64).sum(
            axis=0).astype(np.float32)
    return out
